# revision 8
# baseline (speedup 1.0000x reference)
"""Trainium2 Bass kernel for nn_PoseLoss: batch-parallel over 8 NeuronCores.

Per core: 2 image pairs. Pipeline per pair:
  - exact fp32 distance matrices via ACT Square(Y_bcast - x) + adds
  - rowmin (DVE reduce) + argmin (max_index on min value)
  - indirect-DMA gather of matched-point channels
  - mutual mask (cycle via exact recompute + tolerance, threshold, vis masks)
  - 8-point Gram matrix A (PE matmuls), smallest-eigenvector via repeated
    squaring of (trace*I - A), gt essential matrix, per-pair loss
Output per core: [1,2] = (sum per_pair, sum success). Host sums and divides.
"""
import numpy as np
from contextlib import ExitStack

import concourse.bass as bass
import concourse.bacc as bacc
import concourse.mybir as mybir
import concourse.tile as tile
from concourse.bass_utils import run_bass_kernel_spmd

F32 = mybir.dt.float32
U32 = mybir.dt.uint32
AF = mybir.ActivationFunctionType
OP = mybir.AluOpType
AX = mybir.AxisListType

B, N, PB = 16, 1024, 2      # total batches, points, batches per core
NT = N // 128               # 8 i-tiles
NCORES = B // PB
CYC_TOL = 1e-3
N_SQ = 17                   # squarings
NORM_ITERS = (0, 4, 8, 12, 15)

_CACHE = {}


def _consts():
    c = {}
    c['ones8'] = np.ones((128, 8), np.float32)
    blkP = np.zeros((41, 2), np.float32)
    blkP[0:9, 0] = 1.0
    blkP[32:41, 1] = 1.0
    c['blkP'] = blkP
    c['blkPT'] = blkP.T.copy()
    eye41 = np.zeros((41, 41), np.float32)
    for i in range(9):
        eye41[i, i] = 1.0
        eye41[32 + i, 32 + i] = 1.0
    c['eye41'] = eye41
    rv = np.zeros((41, 1), np.float32)
    r9 = np.cos(np.arange(1, 10, dtype=np.float32) * np.float32(1.7)).astype(np.float32)
    rv[0:9, 0] = r9
    rv[32:41, 0] = r9
    c['rvec'] = rv
    selE8 = np.zeros((41, 2), np.float32)
    selE8[8, 0] = 1.0
    selE8[40, 1] = 1.0
    c['selE8'] = selE8
    s90 = np.zeros((10, 2), np.float32); s90[9, 0] = 1.0
    s91 = np.zeros((10, 2), np.float32); s91[9, 1] = 1.0
    c['sel9_b0'] = s90
    c['sel9_b1'] = s91
    c['ones2'] = np.ones((2, 1), np.float32)
    return c


def _build():
    nc = bacc.Bacc("TRN2", target_bir_lowering=False, debug=False)
    dt = F32

    def din(name, shape):
        return nc.dram_tensor(name, list(shape), dt, kind="ExternalInput")

    kp1_d = din("kp1", (PB, N, 2))
    kp2_d = din("kp2", (PB, N, 2))
    wk1_d = din("wk1", (PB, N, 2))
    wk2_d = din("wk2", (PB, N, 2))
    m1_d = din("m1", (PB, N))
    m2_d = din("m2", (PB, N))
    ss1_d = din("ss1", (PB, 4))
    ss2_d = din("ss2", (PB, 4))
    k1_d = din("K1", (PB, 3, 3))
    k2_d = din("K2", (PB, 3, 3))
    e1_d = din("ext1", (PB, 4, 4))
    e2_d = din("ext2", (PB, 4, 4))
    cn = {k: din(k, v.shape) for k, v in _consts().items()}
    out_d = nc.dram_tensor("out", [1, 2], dt, kind="ExternalOutput")

    with tile.TileContext(nc) as tc, ExitStack() as ctx:
        const = ctx.enter_context(tc.tile_pool(name="const", bufs=1))
        small = ctx.enter_context(tc.tile_pool(name="small", bufs=2))
        big = ctx.enter_context(tc.tile_pool(name="big", bufs=2))
        dbuf = ctx.enter_context(tc.tile_pool(name="dbuf", bufs=2))
        psum = ctx.enter_context(tc.tile_pool(name="psum", bufs=2, space="PSUM"))
        psA = ctx.enter_context(tc.tile_pool(name="psA", bufs=1, space="PSUM"))
        dram = ctx.enter_context(tc.tile_pool(name="dram", bufs=2, space="DRAM"))

        def ld(pool, src_ap, shape, dtype=dt, tag=None):
            t = pool.tile(list(shape), dtype, tag=tag)
            nc.sync.dma_start(t[:], src_ap)
            return t

        C = {k: ld(const, cn[k].ap(), cn[k].shape, tag=k) for k in cn}

        # ---------- per-batch big matching phase ----------
        ppair_sb = small.tile([2, 1], dt, tag="ppair")   # filled per batch later
        sumw_stage = small.tile([10, 2], dt, tag="sumwstage")
        A10_sb = [None, None]
        Wq = [None, None]

        for b in range(PB):
            # ---- load per-batch point data ----
            # queries in i-layout [128, (t c)]
            def q_load(d, tag):
                ap = d.ap()[b].rearrange("(t p) c -> p t c", p=128)
                t = big.tile([128, NT, 2], dt, tag=tag)
                nc.sync.dma_start(t[:], ap)
                return t

            wk1_q = q_load(wk1_d, f"wk1q{b}")
            wk2_q = q_load(wk2_d, f"wk2q{b}")
            kp1_q = q_load(kp1_d, f"kp1q{b}")
            m1_q = big.tile([128, NT], dt, tag=f"m1q{b}")
            nc.sync.dma_start(m1_q[:], m1_d.ap()[b].rearrange("(t p) -> p t", p=128))

            # negated queries for ACT bias
            wk1_n = big.tile([128, NT, 2], dt, tag=f"wk1n{b}")
            nc.vector.tensor_scalar(out=wk1_n[:], in0=wk1_q[:], scalar1=-1.0,
                                    scalar2=None, op0=OP.mult)
            wk2_n = big.tile([128, NT, 2], dt, tag=f"wk2n{b}")
            nc.vector.tensor_scalar(out=wk2_n[:], in0=wk2_q[:], scalar1=-1.0,
                                    scalar2=None, op0=OP.mult)

            # DB rows broadcast [128, N] per coord: d1 db=kp2, d2 db=kp1
            def db_bcast(d, c, tag):
                row = small.tile([1, N], dt, tag=f"row{tag}")
                nc.sync.dma_start(row[:], d.ap()[b][:, c:c + 1].rearrange("n o -> o n"))
                bc = big.tile([128, N], dt, tag=f"bc{tag}")
                nc.gpsimd.partition_broadcast(bc[:], row[:], channels=128)
                return bc

            y0b_1 = db_bcast(kp2_d, 0, f"d1x{b}")
            y1b_1 = db_bcast(kp2_d, 1, f"d1y{b}")
            y0b_2 = db_bcast(kp1_d, 0, f"d2x{b}")
            y1b_2 = db_bcast(kp1_d, 1, f"d2y{b}")

            m1min = big.tile([128, NT], dt, tag=f"m1min{b}")
            m2min = big.tile([128, NT], dt, tag=f"m2min{b}")
            idx_all = big.tile([128, NT], U32, tag=f"idx{b}")

            # d1 side: distances, rowmin, argmin
            for t in range(NT):
                T0 = dbuf.tile([128, N], dt, tag="T0")
                nc.scalar.activation(T0[:], y0b_1[:], AF.Square,
                                     bias=wk1_n[:, t, 0:1], scale=1.0)
                T1 = dbuf.tile([128, N], dt, tag="T1")
                nc.scalar.activation(T1[:], y1b_1[:], AF.Square,
                                     bias=wk1_n[:, t, 1:2], scale=1.0)
                D = dbuf.tile([128, N], dt, tag="D1")
                nc.gpsimd.tensor_tensor(out=D[:], in0=T0[:], in1=T1[:], op=OP.add)
                mt = small.tile([128, 1], dt, tag="mt")
                nc.vector.tensor_reduce(out=mt[:], in_=D[:], axis=AX.X, op=OP.min)
                nc.vector.tensor_copy(m1min[:, t:t + 1], mt[:])
                m8 = small.tile([128, 8], dt, tag="m8")
                nc.vector.tensor_scalar(out=m8[:], in0=C['ones8'][:], scalar1=mt[:],
                                        scalar2=None, op0=OP.mult)
                i8 = small.tile([128, 8], U32, tag="i8")
                nc.vector.max_index(out=i8[:], in_max=m8[:], in_values=D[:])
                nc.vector.tensor_copy(idx_all[:, t:t + 1], i8[:, 0:1])

            # d2 side: distances + rowmin only
            for t in range(NT):
                T0 = dbuf.tile([128, N], dt, tag="T0")
                nc.scalar.activation(T0[:], y0b_2[:], AF.Square,
                                     bias=wk2_n[:, t, 0:1], scale=1.0)
                T1 = dbuf.tile([128, N], dt, tag="T1")
                nc.scalar.activation(T1[:], y1b_2[:], AF.Square,
                                     bias=wk2_n[:, t, 1:2], scale=1.0)
                D = dbuf.tile([128, N], dt, tag="D2")
                nc.vector.tensor_tensor(out=D[:], in0=T0[:], in1=T1[:], op=OP.add)
                mt2 = small.tile([128, 1], dt, tag="mt2")
                nc.vector.tensor_reduce(out=mt2[:], in_=D[:], axis=AX.X, op=OP.min)
                nc.vector.tensor_copy(m2min[:, t:t + 1], mt2[:])

            # ---- gather table -> DRAM, then indirect gather at idx ----
            stage = big.tile([128, NT, 8], dt, tag=f"stage{b}")
            nc.vector.memset(stage[:], 0.0)
            nc.sync.dma_start(stage[:, :, 0:2],
                              kp2_d.ap()[b].rearrange("(t p) c -> p t c", p=128))
            nc.sync.dma_start(stage[:, :, 2:4],
                              wk2_d.ap()[b].rearrange("(t p) c -> p t c", p=128))
            nc.sync.dma_start(stage[:, :, 4],
                              m2_d.ap()[b].rearrange("(t p) -> p t", p=128))
            nc.vector.tensor_copy(stage[:, :, 5], m2min[:])

            table = dram.tile([N, 8], dt, tag="table")
            nc.sync.dma_start(
                table[:].rearrange("(t p) c -> p t c", p=128), stage[:])
            G2 = big.tile([128, NT * 8], dt, tag=f"G{b}")
            for t in range(NT):
                nc.gpsimd.indirect_dma_start(
                    out=G2[:, 8 * t:8 * t + 8], out_offset=None, in_=table[:],
                    in_offset=bass.IndirectOffsetOnAxis(ap=idx_all[:, t:t + 1], axis=0))
            G = G2[:].rearrange("p (t c) -> p t c", c=8)

            # ---- per-batch scalar constants ----
            k1r = small.tile([1, 9], dt, tag="k1r")
            nc.sync.dma_start(k1r[:], k1_d.ap()[b:b + 1].rearrange("b i j -> b (i j)"))
            k2r = small.tile([1, 9], dt, tag="k2r")
            nc.sync.dma_start(k2r[:], k2_d.ap()[b:b + 1].rearrange("b i j -> b (i j)"))
            s1r = small.tile([1, 4], dt, tag="s1r")
            nc.sync.dma_start(s1r[:], ss1_d.ap()[b:b + 1])
            s2r = small.tile([1, 4], dt, tag="s2r")
            nc.sync.dma_start(s2r[:], ss2_d.ap()[b:b + 1])

            sc = small.tile([1, 8], dt, tag="sc")  # u1s u1o v1s v1o u2s u2o v2s v2o
            rf1 = small.tile([1, 1], dt, tag="rf1")
            nc.vector.reciprocal(rf1[:], k1r[:, 0:1])
            rf2 = small.tile([1, 1], dt, tag="rf2")
            nc.vector.reciprocal(rf2[:], k2r[:, 0:1])
            tmp = small.tile([1, 2], dt, tag="sctmp")
            # u1s = ss1[2]*rf1 ; v1s = ss1[3]*rf1
            nc.vector.tensor_scalar(out=sc[:, 0:1], in0=s1r[:, 2:3], scalar1=rf1[:, 0:1],
                                    scalar2=None, op0=OP.mult)
            nc.vector.tensor_scalar(out=sc[:, 2:3], in0=s1r[:, 3:4], scalar1=rf1[:, 0:1],
                                    scalar2=None, op0=OP.mult)
            # u1o = (ss1[0]-cx1)*rf1 ; v1o = (ss1[1]-cy1)*rf1
            nc.vector.tensor_tensor(out=tmp[:, 0:1], in0=s1r[:, 0:1], in1=k1r[:, 2:3], op=OP.subtract)
            nc.vector.tensor_tensor(out=tmp[:, 1:2], in0=s1r[:, 1:2], in1=k1r[:, 5:6], op=OP.subtract)
            nc.vector.tensor_scalar(out=sc[:, 1:2], in0=tmp[:, 0:1], scalar1=rf1[:, 0:1],
                                    scalar2=None, op0=OP.mult)
            nc.vector.tensor_scalar(out=sc[:, 3:4], in0=tmp[:, 1:2], scalar1=rf1[:, 0:1],
                                    scalar2=None, op0=OP.mult)
            # u2s = ss2[2]*rf2 ; v2s = ss2[3]*rf2 ; u2o = (ss2[0]-cx2)*rf2 ; v2o = (ss2[1]-cy2)*rf2
            nc.vector.tensor_scalar(out=sc[:, 4:5], in0=s2r[:, 2:3], scalar1=rf2[:, 0:1],
                                    scalar2=None, op0=OP.mult)
            nc.vector.tensor_scalar(out=sc[:, 6:7], in0=s2r[:, 3:4], scalar1=rf2[:, 0:1],
                                    scalar2=None, op0=OP.mult)
            tmp2 = small.tile([1, 2], dt, tag="sctmp2")
            nc.vector.tensor_tensor(out=tmp2[:, 0:1], in0=s2r[:, 0:1], in1=k2r[:, 2:3], op=OP.subtract)
            nc.vector.tensor_tensor(out=tmp2[:, 1:2], in0=s2r[:, 1:2], in1=k2r[:, 5:6], op=OP.subtract)
            nc.vector.tensor_scalar(out=sc[:, 5:6], in0=tmp2[:, 0:1], scalar1=rf2[:, 0:1],
                                    scalar2=None, op0=OP.mult)
            nc.vector.tensor_scalar(out=sc[:, 7:8], in0=tmp2[:, 1:2], scalar1=rf2[:, 0:1],
                                    scalar2=None, op0=OP.mult)
            scb = small.tile([128, 8], dt, tag="scb")
            nc.gpsimd.partition_broadcast(scb[:], sc[:], channels=128)

            # ---- per-i elementwise ----
            kp1x = kp1_q[:, :, 0]
            kp1y = kp1_q[:, :, 1]
            gkx = G[:, :, 0]; gky = G[:, :, 1]
            gwx = G[:, :, 2]; gwy = G[:, :, 3]
            gm2 = G[:, :, 4]; gn2 = G[:, :, 5]

            t8a = big.tile([128, NT], dt, tag="t8a")
            t8b = big.tile([128, NT], dt, tag="t8b")
            dck = big.tile([128, NT], dt, tag="dck")
            nc.vector.tensor_tensor(out=t8a[:], in0=gwx, in1=kp1x, op=OP.subtract)
            nc.vector.tensor_tensor(out=t8a[:], in0=t8a[:], in1=t8a[:], op=OP.mult)
            nc.vector.tensor_tensor(out=t8b[:], in0=gwy, in1=kp1y, op=OP.subtract)
            nc.vector.tensor_tensor(out=t8b[:], in0=t8b[:], in1=t8b[:], op=OP.mult)
            nc.vector.tensor_tensor(out=dck[:], in0=t8a[:], in1=t8b[:], op=OP.add)
            # cycle: dck <= gn2 + tol
            cyc = big.tile([128, NT], dt, tag="cyc")
            nc.vector.tensor_scalar(out=cyc[:], in0=gn2, scalar1=float(CYC_TOL),
                                    scalar2=None, op0=OP.add)
            nc.vector.tensor_tensor(out=cyc[:], in0=dck[:], in1=cyc[:], op=OP.is_le)
            # w = cyc * (m1min<=9) * m1 * gm2
            w = big.tile([128, NT], dt, tag=f"w{b}")
            nc.vector.tensor_scalar(out=w[:], in0=m1min[:], scalar1=9.0,
                                    scalar2=None, op0=OP.is_le)
            nc.vector.tensor_tensor(out=w[:], in0=w[:], in1=cyc[:], op=OP.mult)
            nc.vector.tensor_tensor(out=w[:], in0=w[:], in1=m1_q[:], op=OP.mult)
            nc.vector.tensor_tensor(out=w[:], in0=w[:], in1=gm2, op=OP.mult)
            Wq[b] = w

            # normalized coords
            u1 = big.tile([128, NT], dt, tag="u1")
            v1 = big.tile([128, NT], dt, tag="v1")
            u2 = big.tile([128, NT], dt, tag="u2")
            v2 = big.tile([128, NT], dt, tag="v2")
            nc.vector.tensor_scalar(out=u1[:], in0=kp1x, scalar1=scb[:, 0:1],
                                    scalar2=scb[:, 1:2], op0=OP.mult, op1=OP.add)
            nc.vector.tensor_scalar(out=v1[:], in0=kp1y, scalar1=scb[:, 2:3],
                                    scalar2=scb[:, 3:4], op0=OP.mult, op1=OP.add)
            nc.vector.tensor_scalar(out=u2[:], in0=gkx, scalar1=scb[:, 4:5],
                                    scalar2=scb[:, 5:6], op0=OP.mult, op1=OP.add)
            nc.vector.tensor_scalar(out=v2[:], in0=gky, scalar1=scb[:, 6:7],
                                    scalar2=scb[:, 7:8], op0=OP.mult, op1=OP.add)

            a_all = big.tile([128, NT, 10], dt, tag=f"a{b}")
            nc.vector.memset(a_all[:], 1.0)
            nc.vector.tensor_tensor(out=a_all[:, :, 0], in0=u2[:], in1=u1[:], op=OP.mult)
            nc.vector.tensor_tensor(out=a_all[:, :, 1], in0=u2[:], in1=v1[:], op=OP.mult)
            nc.vector.tensor_copy(a_all[:, :, 2], u2[:])
            nc.vector.tensor_tensor(out=a_all[:, :, 3], in0=v2[:], in1=u1[:], op=OP.mult)
            nc.vector.tensor_tensor(out=a_all[:, :, 4], in0=v2[:], in1=v1[:], op=OP.mult)
            nc.vector.tensor_copy(a_all[:, :, 5], v2[:])
            nc.vector.tensor_copy(a_all[:, :, 6], u1[:])
            nc.vector.tensor_copy(a_all[:, :, 7], v1[:])
            # col 8,9 stay 1.0

            wa_all = big.tile([128, NT, 10], dt, tag=f"wa{b}")
            for t in range(NT):
                nc.vector.tensor_scalar(out=wa_all[:, t, :], in0=a_all[:, t, :],
                                        scalar1=w[:, t:t + 1], scalar2=None, op0=OP.mult)

            A10 = psA.tile([10, 10], dt, tag=f"A10_{b}")
            for t in range(NT):
                nc.tensor.matmul(A10[:], wa_all[:, t, :], a_all[:, t, :],
                                 start=(t == 0), stop=(t == NT - 1))
            A10s = small.tile([10, 10], dt, tag=f"A10s{b}")
            nc.scalar.activation(A10s[:], A10[:], AF.Copy)
            A10_sb[b] = A10s
            nc.scalar.activation(sumw_stage[:, b:b + 1], A10[:, 9:10], AF.Copy)

        # ---------- eigen phase (both batches block-diag in [41,41]) ----------
        Ablk = small.tile([41, 41], dt, tag="Ablk")
        nc.vector.memset(Ablk[:], 0.0)
        nc.scalar.activation(Ablk[0:9, 0:9], A10_sb[0][0:9, 0:9], AF.Copy)
        nc.scalar.activation(Ablk[32:41, 32:41], A10_sb[1][0:9, 0:9], AF.Copy)

        diag = small.tile([41, 41], dt, tag="diagm")
        nc.vector.tensor_tensor(out=diag[:], in0=Ablk[:], in1=C['eye41'][:], op=OP.mult)
        rs = small.tile([41, 1], dt, tag="rs41")
        nc.vector.tensor_reduce(out=rs[:], in_=diag[:], axis=AX.X, op=OP.add)
        tr2 = psum.tile([2, 1], dt, tag="ps")
        nc.tensor.matmul(tr2[:], C['blkP'][:], rs[:], start=True, stop=True)
        tr2s = small.tile([2, 1], dt, tag="tr2s")
        nc.scalar.activation(tr2s[:], tr2[:], AF.Copy)
        s41p = psum.tile([41, 1], dt, tag="ps")
        nc.tensor.matmul(s41p[:], C['blkPT'][:], tr2s[:], start=True, stop=True)
        s41s = small.tile([41, 1], dt, tag="s41s")
        nc.scalar.activation(s41s[:], s41p[:], AF.Copy)
        M = small.tile([41, 41], dt, tag="M")
        nc.vector.tensor_scalar(out=M[:], in0=C['eye41'][:], scalar1=s41s[:],
                                scalar2=None, op0=OP.mult)
        nc.vector.tensor_tensor(out=M[:], in0=M[:], in1=Ablk[:], op=OP.subtract)

        for it in range(N_SQ):
            if it in NORM_ITERS:
                sq = small.tile([41, 41], dt, tag="sqM")
                nc.vector.tensor_tensor(out=sq[:], in0=M[:], in1=M[:], op=OP.mult)
                rsn = small.tile([41, 1], dt, tag="rsn")
                nc.vector.tensor_reduce(out=rsn[:], in_=sq[:], axis=AX.X, op=OP.add)
                f2 = psum.tile([2, 1], dt, tag="ps")
                nc.tensor.matmul(f2[:], C['blkP'][:], rsn[:], start=True, stop=True)
                rinv = small.tile([2, 1], dt, tag="rinv")
                nc.vector.reciprocal(rinv[:], f2[:])
                rsq = small.tile([2, 1], dt, tag="rsq")
                nc.scalar.activation(rsq[:], rinv[:], AF.Sqrt)
                sb41 = psum.tile([41, 1], dt, tag="ps")
                nc.tensor.matmul(sb41[:], C['blkPT'][:], rsq[:], start=True, stop=True)
                sb41s = small.tile([41, 1], dt, tag="sb41s")
                nc.scalar.activation(sb41s[:], sb41[:], AF.Copy)
                Mn = small.tile([41, 41], dt, tag="Mn")
                nc.vector.tensor_scalar(out=Mn[:], in0=M[:], scalar1=sb41s[:],
                                        scalar2=None, op0=OP.mult)
                M = Mn
            Msq = psum.tile([41, 41], dt, tag="Msq")
            nc.tensor.matmul(Msq[:], M[:], M[:], start=True, stop=True)
            M2 = small.tile([41, 41], dt, tag="M")
            nc.scalar.activation(M2[:], Msq[:], AF.Copy)
            M = M2

        vps = psum.tile([41, 1], dt, tag="ps")
        nc.tensor.matmul(vps[:], M[:], C['rvec'][:], start=True, stop=True)
        vs = small.tile([41, 1], dt, tag="vs")
        nc.scalar.activation(vs[:], vps[:], AF.Copy)
        vsq = small.tile([41, 1], dt, tag="vsq")
        nc.vector.tensor_tensor(out=vsq[:], in0=vs[:], in1=vs[:], op=OP.mult)
        n2 = psum.tile([2, 1], dt, tag="ps")
        nc.tensor.matmul(n2[:], C['blkP'][:], vsq[:], start=True, stop=True)
        n2i = small.tile([2, 1], dt, tag="n2i")
        nc.vector.reciprocal(n2i[:], n2[:])
        n2r = small.tile([2, 1], dt, tag="n2r")
        nc.scalar.activation(n2r[:], n2i[:], AF.Sqrt)
        nb41 = psum.tile([41, 1], dt, tag="ps")
        nc.tensor.matmul(nb41[:], C['blkPT'][:], n2r[:], start=True, stop=True)
        nb41s = small.tile([41, 1], dt, tag="nb41s")
        nc.scalar.activation(nb41s[:], nb41[:], AF.Copy)
        e41 = small.tile([41, 1], dt, tag="e41")
        nc.vector.tensor_scalar(out=e41[:], in0=vs[:], scalar1=nb41s[:],
                                scalar2=None, op0=OP.mult)
        # sign fix via e[8]
        e8 = psum.tile([2, 1], dt, tag="ps")
        nc.tensor.matmul(e8[:], C['selE8'][:], e41[:], start=True, stop=True)
        sg = small.tile([2, 1], dt, tag="sg")
        nc.scalar.activation(sg[:], e8[:], AF.Sign)
        sg41 = psum.tile([41, 1], dt, tag="ps")
        nc.tensor.matmul(sg41[:], C['blkPT'][:], sg[:], start=True, stop=True)
        sg41s = small.tile([41, 1], dt, tag="sg41s")
        nc.scalar.activation(sg41s[:], sg41[:], AF.Copy)
        nc.vector.tensor_scalar(out=e41[:], in0=e41[:], scalar1=sg41s[:],
                                scalar2=None, op0=OP.mult)

        # ---------- gt essential matrix, per-batch in [2,*] layout ----------
        R2d = small.tile([2, 9], dt, tag="R2d")
        t2d = small.tile([2, 3], dt, tag="t2d")
        for b in range(PB):
            e1T = small.tile([4, 4], dt, tag="e1T")
            nc.sync.dma_start(e1T[:], e1_d.ap()[b].rearrange("i j -> j i"))
            e2T = small.tile([4, 4], dt, tag="e2T")
            nc.sync.dma_start(e2T[:], e2_d.ap()[b].rearrange("i j -> j i"))
            Rps = psum.tile([4, 4], dt, tag="ps")
            nc.tensor.matmul(Rps[:], e2T[0:3, :], e1T[0:3, :], start=True, stop=True)
            Rs = small.tile([4, 4], dt, tag="Rs")
            nc.scalar.activation(Rs[:], Rps[:], AF.Copy)
            RTps = psum.tile([4, 4], dt, tag="ps")
            nc.tensor.matmul(RTps[:], e1T[0:3, :], e2T[0:3, :], start=True, stop=True)
            RTs = small.tile([4, 4], dt, tag="RTs")
            nc.scalar.activation(RTs[:], RTps[:], AF.Copy)
            t1s = small.tile([3, 1], dt, tag="t1s")
            nc.sync.dma_start(t1s[:], e1_d.ap()[b][0:3, 3:4])
            t2s = small.tile([3, 1], dt, tag="t2s")
            nc.sync.dma_start(t2s[:], e2_d.ap()[b][0:3, 3:4])
            Rt1 = psum.tile([4, 1], dt, tag="ps")
            nc.tensor.matmul(Rt1[:], RTs[0:3, 0:4], t1s[:], start=True, stop=True)
            tv = small.tile([3, 1], dt, tag="tv")
            nc.vector.tensor_tensor(out=tv[:], in0=t2s[:], in1=Rt1[0:3, :], op=OP.subtract)
            nc.sync.dma_start(R2d[b:b + 1, :], Rs[0:3, 0:3])
            nc.sync.dma_start(t2d[b:b + 1, :], tv[:])

        # E = skew(t) @ R in [2,9] layout
        E2d = small.tile([2, 9], dt, tag="E2d")
        tmpE = small.tile([2, 3], dt, tag="tmpE")
        # row0 = -t2*R[1,:] + t1*R[2,:]
        nc.vector.tensor_scalar(out=tmpE[:], in0=R2d[:, 3:6], scalar1=t2d[:, 2:3],
                                scalar2=-1.0, op0=OP.mult, op1=OP.mult)
        nc.vector.tensor_scalar(out=E2d[:, 0:3], in0=R2d[:, 6:9], scalar1=t2d[:, 1:2],
                                scalar2=None, op0=OP.mult)
        nc.vector.tensor_tensor(out=E2d[:, 0:3], in0=E2d[:, 0:3], in1=tmpE[:], op=OP.add)
        # row1 = t2*R[0,:] - t0*R[2,:]
        nc.vector.tensor_scalar(out=tmpE[:], in0=R2d[:, 6:9], scalar1=t2d[:, 0:1],
                                scalar2=-1.0, op0=OP.mult, op1=OP.mult)
        nc.vector.tensor_scalar(out=E2d[:, 3:6], in0=R2d[:, 0:3], scalar1=t2d[:, 2:3],
                                scalar2=None, op0=OP.mult)
        nc.vector.tensor_tensor(out=E2d[:, 3:6], in0=E2d[:, 3:6], in1=tmpE[:], op=OP.add)
        # row2 = -t1*R[0,:] + t0*R[1,:]
        nc.vector.tensor_scalar(out=tmpE[:], in0=R2d[:, 0:3], scalar1=t2d[:, 1:2],
                                scalar2=-1.0, op0=OP.mult, op1=OP.mult)
        nc.vector.tensor_scalar(out=E2d[:, 6:9], in0=R2d[:, 3:6], scalar1=t2d[:, 0:1],
                                scalar2=None, op0=OP.mult)
        nc.vector.tensor_tensor(out=E2d[:, 6:9], in0=E2d[:, 6:9], in1=tmpE[:], op=OP.add)

        gsq = small.tile([2, 9], dt, tag="gsq")
        nc.vector.tensor_tensor(out=gsq[:], in0=E2d[:], in1=E2d[:], op=OP.mult)
        gn = small.tile([2, 1], dt, tag="gn")
        nc.vector.tensor_reduce(out=gn[:], in_=gsq[:], axis=AX.X, op=OP.add)
        gns = small.tile([2, 1], dt, tag="gns")
        nc.scalar.activation(gns[:], gn[:], AF.Sqrt)
        nc.vector.tensor_scalar(out=gns[:], in0=gns[:], scalar1=1e-8,
                                scalar2=None, op0=OP.max)
        gni = small.tile([2, 1], dt, tag="gni")
        nc.vector.reciprocal(gni[:], gns[:])
        g2d = small.tile([2, 9], dt, tag="g2d")
        nc.vector.tensor_scalar(out=g2d[:], in0=E2d[:], scalar1=gni[:],
                                scalar2=None, op0=OP.mult)
        sgg = small.tile([2, 1], dt, tag="sgg")
        nc.scalar.activation(sgg[:], g2d[:, 8:9], AF.Sign)
        nc.vector.tensor_scalar(out=g2d[:], in0=g2d[:], scalar1=sgg[:],
                                scalar2=None, op0=OP.mult)
        g41 = small.tile([41, 1], dt, tag="g41")
        nc.vector.memset(g41[:], 0.0)
        nc.sync.dma_start(g41[0:9, :], g2d[0:1, :])
        nc.sync.dma_start(g41[32:41, :], g2d[1:2, :])

        # ---------- per-pair loss ----------
        dif = small.tile([41, 1], dt, tag="dif")
        nc.vector.tensor_tensor(out=dif[:], in0=e41[:], in1=g41[:], op=OP.subtract)
        nc.vector.tensor_tensor(out=dif[:], in0=dif[:], in1=dif[:], op=OP.mult)
        pp2 = psum.tile([2, 1], dt, tag="ps")
        nc.tensor.matmul(pp2[:], C['blkP'][:], dif[:], start=True, stop=True)
        pp = small.tile([2, 1], dt, tag="pp")
        nc.scalar.activation(pp[:], pp2[:], AF.Sqrt)

        sumw2 = psA.tile([2, 1], dt, tag="sumw2tag")
        nc.tensor.matmul(sumw2[:], C['sel9_b0'][:], sumw_stage[:, 0:1], start=True, stop=False)
        nc.tensor.matmul(sumw2[:], C['sel9_b1'][:], sumw_stage[:, 1:2], start=False, stop=True)
        succ = small.tile([2, 1], dt, tag="succ")
        nc.vector.tensor_scalar(out=succ[:], in0=sumw2[:], scalar1=8.0,
                                scalar2=None, op0=OP.is_ge)
        numden = small.tile([2, 2], dt, tag="numden")
        nc.vector.tensor_tensor(out=numden[:, 0:1], in0=pp[:], in1=succ[:], op=OP.mult)
        nc.vector.tensor_copy(numden[:, 1:2], succ[:])
        outp = psum.tile([1, 2], dt, tag="ps")
        ones2t = small.tile([2, 1], dt, tag="ones2t")
        nc.vector.memset(ones2t[:], 1.0)
        nc.tensor.matmul(outp[:], ones2t[:], numden[:], start=True, stop=True)
        outs = small.tile([1, 2], dt, tag="outs")
        nc.scalar.activation(outs[:], outp[:], AF.Copy)
        nc.sync.dma_start(out_d.ap(), outs[:])

    nc.compile()
    return nc


def _in_maps(inputs):
    cns = _consts()
    maps = []
    f32 = np.float32
    for c in range(NCORES):
        sl = slice(c * PB, (c + 1) * PB)
        m = {
            "kp1": np.ascontiguousarray(inputs['kp1'][sl], f32),
            "kp2": np.ascontiguousarray(inputs['kp2'][sl], f32),
            "wk1": np.ascontiguousarray(inputs['w_kp1'][sl], f32),
            "wk2": np.ascontiguousarray(inputs['w_kp2'][sl], f32),
            "m1": np.ascontiguousarray(inputs['w_vis_kp1_mask'][sl], f32),
            "m2": np.ascontiguousarray(inputs['w_vis_kp2_mask'][sl], f32),
            "ss1": np.ascontiguousarray(inputs['shift_scale1'][sl], f32),
            "ss2": np.ascontiguousarray(inputs['shift_scale2'][sl], f32),
            "K1": np.ascontiguousarray(inputs['intrinsics1'][sl], f32),
            "K2": np.ascontiguousarray(inputs['intrinsics2'][sl], f32),
            "ext1": np.ascontiguousarray(inputs['extrinsics1'][sl], f32),
            "ext2": np.ascontiguousarray(inputs['extrinsics2'][sl], f32),
        }
        m.update({k: v for k, v in cns.items()})
        maps.append(m)
    return maps


def kernel(**inputs):
    if 'nc' not in _CACHE:
        _CACHE['nc'] = _build()
    nc = _CACHE['nc']
    res = run_bass_kernel_spmd(nc, _in_maps(inputs), core_ids=list(range(NCORES)))
    num = np.float32(0.0)
    den = np.float32(0.0)
    for c in range(NCORES):
        o = np.asarray(res.results[c]["out"], np.float32)
        num = np.float32(num + o[0, 0])
        den = np.float32(den + o[0, 1])
    loss = np.float32(num / max(den, np.float32(1e-8)))
    return np.asarray(loss, np.float32)


if __name__ == "__main__":
    d = np.load('/tmp/inputs.npz')
    out = kernel(**{k: d[k] for k in d.files})
    print("kernel loss:", out)
    print("ref loss:", np.load('/tmp/ref_loss.npy'))
